# revision 1
# baseline (speedup 1.0000x reference)
"""MoDL (CNN denoiser + CG data-consistency MRI recon) on 8 Trainium2 NeuronCores.

Sharding: data-parallel over batch B=8, one batch element per core. The CG
dot-products are global over the batch in the reference, so each CG iteration
does two tiny AllReduce collectives (denom, rTrNew) across the 8 cores.

All FFTs are direct DFTs as PE matmuls (fp32r = FP22 mantissa, full speed at
free-dim>=256). Each 1-D DFT stage uses the DATA as the stationary operand,
which makes every stage output land pre-transposed for the next stage
("tall-split" layout [128 partitions, 2 tiles x 256] throughout) - no
explicit transposes anywhere.

CNN: channels-on-partitions, shifted-window matmuls; 3x3 offsets are packed
in pairs along the contraction axis (partitions 64..127 hold a one-column-
shifted copy of the strip) so middle layers run 6 matmul groups instead of 9.
"""

from contextlib import ExitStack

import numpy as np

import concourse.bass as bass
import concourse.tile as tile
from concourse import bacc
from concourse import mybir
from concourse.bass_utils import run_bass_kernel_spmd
from concourse import bass_isa

FP = mybir.dt.float32
FPR = mybir.dt.float32r
AX = mybir.AxisListType
OP = mybir.AluOpType
AF = mybir.ActivationFunctionType

B, NCOIL, H, W = 8, 12, 256, 256
N_CG = 11
N_CORES = 8
HW = H * W


# ---------------------------------------------------------------- host prep

def _tall_split(plane):
    """[256,256] -> [128, 512]: col block t holds rows t*128..t*128+127."""
    return np.ascontiguousarray(
        plane.reshape(2, 128, 256).transpose(1, 0, 2).reshape(128, 512))


def _make_fmats():
    n = np.arange(256)
    Fm = np.exp(-2j * np.pi * np.outer(n, n) / 256) / 16.0
    fr = Fm.real.astype(np.float32)
    fi = Fm.imag.astype(np.float32)
    return np.stack([_tall_split(fr), _tall_split(fi), _tall_split(-fi)])


def _prep_shared(w1, b1, w2, b2, w3, b3, w4, b4, w5, b5, lam):
    out = {}
    out["fmats"] = _make_fmats()
    out["wpack1"] = np.ascontiguousarray(
        np.asarray(w1, np.float32).transpose(2, 3, 1, 0).reshape(18, 64))

    def pack(wl):
        wl = np.asarray(wl, np.float32)
        cout = wl.shape[0]
        g = np.zeros((6, 128, cout), np.float32)
        for idy in range(3):
            g[idy * 2 + 0, 0:64] = wl[:, :, idy, 0].T    # dx=-1
            g[idy * 2 + 0, 64:128] = wl[:, :, idy, 1].T  # dx=0
            g[idy * 2 + 1, 0:64] = wl[:, :, idy, 2].T    # dx=+1
        return g
    out["wpack2"] = pack(w2)
    out["wpack3"] = pack(w3)
    out["wpack4"] = pack(w4)
    out["wpack5"] = pack(w5)
    for i, bl in enumerate((b1, b2, b3, b4, b5)):
        out[f"bias{i + 1}"] = np.asarray(bl, np.float32).reshape(-1, 1)
    out["lam"] = np.asarray(lam, np.float32).reshape(1, 1)
    return out


def _prep_core(atb_b, csm_re_b, csm_im_b, mask_b):
    out = {}
    pad = np.zeros((2, 258, 258), np.float32)
    pad[:, 1:257, 1:257] = atb_b
    im = np.empty((18, 256, 256), np.float32)
    for idy in range(3):
        for idx in range(3):
            o = idy * 3 + idx
            im[2 * o:2 * o + 2] = pad[:, idy:idy + 256, idx:idx + 256]
    out["im2col"] = np.ascontiguousarray(im.reshape(18, HW))
    out["atb_ts"] = np.stack([_tall_split(atb_b[0]), _tall_split(atb_b[1])])
    out["csm_re"] = np.ascontiguousarray(
        np.stack([_tall_split(csm_re_b[c]) for c in range(NCOIL)], axis=1)
        .reshape(128, NCOIL * 512))
    out["csm_im"] = np.ascontiguousarray(
        np.stack([_tall_split(csm_im_b[c]) for c in range(NCOIL)], axis=1)
        .reshape(128, NCOIL * 512))
    out["mask_ts"] = _tall_split(mask_b)
    return out


# ------------------------------------------------------------- bass program

def _r(ap):
    return ap


def build_nc(n_cg=N_CG, n_coil=NCOIL, n_cores=N_CORES, cnn=True, evsem=True,
             use_cc=True):
    _uid = [0]

    def T(pool, shape, tag, dt=FP):
        _uid[0] += 1
        return pool.tile(shape, dt, tag=tag, name=f"{tag}_{_uid[0]}")

    nc = bass.Bass(num_devices=n_cores)
    group = [list(range(n_cores))]

    din = {}
    for name, shape in [
        ("im2col", [18, HW]), ("atb_ts", [2, 128, 512]),
        ("csm_re", [128, n_coil * 512]), ("csm_im", [128, n_coil * 512]),
        ("mask_ts", [128, 512]), ("fmats", [3, 128, 512]),
        ("wpack1", [18, 64]), ("wpack2", [6, 128, 64]),
        ("wpack3", [6, 128, 64]), ("wpack4", [6, 128, 64]),
        ("wpack5", [6, 128, 2]),
        ("bias1", [64, 1]), ("bias2", [64, 1]), ("bias3", [64, 1]),
        ("bias4", [64, 1]), ("bias5", [2, 1]), ("lam", [1, 1]),
    ]:
        din[name] = nc.declare_dram_parameter(name, shape, FP, isOutput=False)
    dout = nc.declare_dram_parameter("out", [2, HW], FP, isOutput=True)

    acta = nc.dram_tensor("acta", [64, 256, 256], FP)
    actb = nc.dram_tensor("actb", [64, 256, 256], FP)
    h5d = nc.dram_tensor("h5d", [2, HW], FP)
    n_cc = 2 * n_cg + 1
    ccin = [nc.dram_tensor(f"ccin{i}", [1, 128], FP) for i in range(n_cc)]
    ccb = [nc.dram_tensor(f"ccb{i}", [1, 1], FP) for i in range(n_cc)]
    cc_space = "Shared" if n_cores > 4 else "Local"
    ccout = [nc.dram_tensor(f"ccout{i}", [1, 128], FP, addr_space=cc_space)
             for i in range(n_cc)]

    with tile.TileContext(nc) as tc, ExitStack() as ctx:  # noqa: SIM117
        consts = ctx.enter_context(tc.tile_pool(name="consts", bufs=1))

        # ---- constants into SBUF ----
        fm = [T(consts, [128, 512], f"fm{j}", FPR) for j in range(3)]
        for j in range(3):
            nc.gpsimd.dma_start(out=fm[j], in_=din["fmats"][j])
        csm_re = T(consts, [128, n_coil * 512], "csm_re")
        csm_im = T(consts, [128, n_coil * 512], "csm_im")
        nc.sync.dma_start(out=csm_re, in_=din["csm_re"][:])
        nc.sync.dma_start(out=csm_im, in_=din["csm_im"][:])
        mask = T(consts, [128, 512], "mask")
        nc.sync.dma_start(out=mask, in_=din["mask_ts"][:])
        lam128 = T(consts, [128, 1], "lam128")
        nc.sync.dma_start(out=lam128, in_=din["lam"][:].to_broadcast([128, 1]))
        ones128 = T(consts, [128, 1], "ones128")
        nc.vector.memset(ones128, 1.0)

        # =========================== CNN ===========================
        if cnn:
            with tc.tile_pool(name="cnnw", bufs=1) as cw, \
                 tc.tile_pool(name="cnnio", bufs=2) as cio, \
                 tc.tile_pool(name="cnnps", bufs=4, space="PSUM") as cps:
                wp = {}
                for l in (2, 3, 4, 5):
                    cout = 2 if l == 5 else 64
                    wp[l] = [T(cw, [128, cout], f"w{l}g{g}", FPR) for g in range(6)]
                    for g in range(6):
                        nc.gpsimd.dma_start(out=wp[l][g], in_=din[f"wpack{l}"][g])
                wp1 = T(cw, [18, 64], "wp1", FPR)
                nc.gpsimd.dma_start(out=wp1, in_=din["wpack1"][:])
                bias = {}
                for l in (1, 2, 3, 4, 5):
                    cout = 2 if l == 5 else 64
                    bias[l] = T(cw, [cout, 1], f"b{l}")
                    nc.sync.dma_start(out=bias[l], in_=din[f"bias{l}"][:])


                # ---- layer 1: K=18 im2col ----
                for s in range(16):
                    mv = T(cio, [18, 16 * 256], "l1mv", FPR)
                    nc.gpsimd.dma_start(
                        out=mv, in_=din["im2col"][:, s * 4096:(s + 1) * 4096])
                    ost = T(cio, [64, 16 * 256], "ostrip")
                    for k in range(8):
                        ps = T(cps, [64, 512], "ps")
                        nc.tensor.matmul(ps, _r(wp1[:]),
                                         _r(mv[:, k * 512:(k + 1) * 512]),
                                         start=True, stop=True)
                        nc.scalar.activation(ost[:, k * 512:(k + 1) * 512], ps,
                                             AF.Relu, bias=bias[1][:])
                    nc.sync.dma_start(
                        out=acta[:, s * 16:(s + 1) * 16, :],
                        in_=ost.rearrange("c (h w) -> c h w", w=256))

                # ---- layers 2..5 ----
                for l, (src, dst) in zip(
                        (2, 3, 4, 5), [(acta, actb), (actb, acta),
                                       (acta, actb), (actb, None)]):
                    cout = 2 if l == 5 else 64
                    for s in range(16):
                        r0 = s * 16
                        ins = T(cio, [128, 18 * 258], "instrip", FPR)
                        ins3 = ins.rearrange("p (r w) -> p r w", r=18)
                        ra = 0 if s > 0 else 1          # first valid ins row
                        rb = 18 if s < 15 else 17       # one past last valid
                        # zero whole tile first (covers padding cols + edge
                        # rows), then DMA the interior on top
                        nc.vector.memset(ins.bitcast(mybir.dt.uint32), 0)
                        nc.gpsimd.dma_start(
                            out=ins3[0:64, ra:rb, 1:257],
                            in_=src[:, r0 - 1 + ra:r0 - 1 + rb, :])
                        nc.gpsimd.dma_start(
                            out=ins3[64:128, ra:rb, 0:256],
                            in_=src[:, r0 - 1 + ra:r0 - 1 + rb, :])
                        ost = T(cio, [cout, 16 * 256], "ostrip")
                        for k in range(8):
                            ps = T(cps, [cout, 512], "ps")
                            for idy in range(3):
                                for half in range(2):
                                    g = idy * 2 + half
                                    dxo = 2 if half else 0
                                    mvap = ins3[:, k * 2 + idy:k * 2 + idy + 2,
                                                dxo:dxo + 256]
                                    nc.tensor.matmul(
                                        ps, _r(wp[l][g]), _r(mvap),
                                        start=(g == 0), stop=(g == 5))
                            nc.scalar.activation(
                                ost[:, k * 512:(k + 1) * 512], ps,
                                AF.Relu if l != 5 else AF.Identity,
                                bias=bias[l][:])
                        if l == 5:
                            nc.sync.dma_start(
                                out=h5d[:, s * 4096:(s + 1) * 4096], in_=ost)
                        else:
                            nc.sync.dma_start(
                                out=dst[:, s * 16:(s + 1) * 16, :],
                                in_=ost.rearrange("c (h w) -> c h w", w=256))

        # =========================== CG ===========================
        st = ctx.enter_context(tc.tile_pool(name="cgstate", bufs=1))
        wk = ctx.enter_context(tc.tile_pool(name="cgwork", bufs=2))
        sc = ctx.enter_context(tc.tile_pool(name="cgsmall", bufs=2))
        pp = ctx.enter_context(tc.tile_pool(name="cgps", bufs=8, space="PSUM"))
        prodp = ctx.enter_context(tc.tile_pool(name="cgprod", bufs=1))

        x_t = [T(st, [128, 512], f"x{i}") for i in range(2)]
        r_t = [T(st, [128, 512], f"r{i}") for i in range(2)]
        p_t = [T(st, [128, 512], f"p{i}") for i in range(2)]
        ap_t = [T(st, [128, 512], f"ap{i}") for i in range(2)]
        accs = [[T(st, [128, 512], f"acc{h}{i}") for i in range(2)]
                for h in range(3)]
        lam1p = T(st, [128, 1], "lam1p")
        nc.vector.tensor_scalar_add(lam1p, lam128, 1.0)

        # rhs = (1+lam)*atb + lam*h5 ; r = p = rhs ; x = 0
        for i in range(2):
            atbp = T(wk, [128, 512], "ld")
            nc.sync.dma_start(out=atbp, in_=din["atb_ts"][i])
            if cnn:
                h5p = T(wk, [128, 512], "ld2")
                nc.sync.dma_start(
                    out=h5p,
                    in_=h5d[i].rearrange("(t p w) -> p t w", t=2, p=128, w=256))
                t0 = T(wk, [128, 512], "ld3")
                nc.vector.tensor_scalar(t0, atbp, lam1p[:], None, op0=OP.mult)
                nc.vector.scalar_tensor_tensor(
                    r_t[i], h5p, lam128[:], t0, op0=OP.mult, op1=OP.add)
            else:
                nc.vector.tensor_scalar(r_t[i], atbp, lam1p[:], None, op0=OP.mult)
            nc.vector.tensor_copy(p_t[i], r_t[i])
            nc.vector.memset(x_t[i], 0.0)

        cc_i = [0]

        def allreduce_scalar(part128):
            """[128,1] per-partition partials -> global scalar bcast [128,1]."""
            i = cc_i[0]
            cc_i[0] += 1
            nc.gpsimd.dma_start(out=ccin[i][:], in_=part128)
            if use_cc:
                nc.gpsimd.collective_compute(
                    "AllReduce", OP.add, replica_groups=group,
                    ins=[ccin[i][:]], outs=[ccout[i][:]])
                src_cc = ccout[i]
            else:
                src_cc = ccin[i]
            g1 = T(sc, [1, 128], "ccg1")
            nc.gpsimd.dma_start(out=g1, in_=src_cc[:])
            gs = T(sc, [1, 1], "ccgs")
            nc.vector.tensor_reduce(gs, g1, axis=AX.X, op=OP.add)
            nc.gpsimd.dma_start(out=ccb[i][:], in_=gs)
            g128 = T(st, [128, 1], f"ccg{i}")
            nc.gpsimd.dma_start(out=g128, in_=ccb[i][:].to_broadcast([128, 1]))
            return g128

        def dot_partial(a_planes, b_planes):
            scrap = T(wk, [128, 512], "dotscrap")
            nc.vector.tensor_tensor(scrap, a_planes[0], b_planes[0], op=OP.mult)
            acc1 = T(sc, [128, 1], "dacc1")
            nc.vector.tensor_reduce(acc1, scrap, axis=AX.X, op=OP.add)
            scrap2 = T(wk, [128, 512], "dotscrap2")
            nc.vector.tensor_tensor(scrap2, a_planes[1], b_planes[1], op=OP.mult)
            acc2 = T(sc, [128, 1], "dacc2")
            nc.vector.tensor_reduce(acc2, scrap2, axis=AX.X, op=OP.add)
            acc3 = T(sc, [128, 1], "dacc3")
            nc.vector.tensor_tensor(acc3, acc1, acc2, op=OP.add)
            return acc3

        def F(j, t):
            return _r(fm[j][:, t * 256:(t + 1) * 256])

        FWD = ((0, 2), (1, 0))   # re: Xr*Fr + Xi*(-Fi); im: Xr*Fi + Xi*Fr
        INV = ((0, 1), (2, 0))   # re: Xr*Fr + Xi*Fi;    im: Xr*(-Fi) + Xi*Fr

        def dft_matmuls(inp, combo):
            ps = [T(pp, [128, 512], "ps") for _ in range(2)]
            for m in range(2):
                for t in range(2):
                    for pl in range(2):
                        lt = _r(inp[pl][:, t * 256 + m * 128:
                                        t * 256 + m * 128 + 128])
                        fst = (t == 0 and pl == 0)
                        lst = (t == 1 and pl == 1)
                        nc.tensor.matmul(ps[0][:, m * 256:(m + 1) * 256], lt,
                                         F(combo[0][pl], t), start=fst, stop=lst)
                        nc.tensor.matmul(ps[1][:, m * 256:(m + 1) * 256], lt,
                                         F(combo[1][pl], t), start=fst, stop=lst)
            return ps

        def eng(c):
            return nc.vector if c % 3 != 2 else nc.gpsimd

        # ---------------- CG loop (unrolled) ----------------
        rtr128 = None
        for it in range(n_cg + 1):
            first = it == 0
            last_iter = it == n_cg
            if not first:
                # --- Ap = AtA(p) ---
                nhalf = (n_coil + 3) // 4
                for half in range(nhalf):
                    prodA = T(prodp, [128, 8 * 512], "pA")
                    prodB = T(prodp, [128, 8 * 512], "pB")
                    ncoil_h = min(4, n_coil - half * 4)
                    for ci in range(ncoil_h):
                        c = half * 4 + ci
                        e = eng(c)
                        cr = csm_re[:, c * 512:(c + 1) * 512]
                        cim = csm_im[:, c * 512:(c + 1) * 512]
                        coil = [T(wk, [128, 512], f"coil{i}", FPR) for i in range(2)]
                        m1 = T(wk, [128, 512], "m1")
                        m2 = T(wk, [128, 512], "m2")
                        e.tensor_tensor(m1, cr, p_t[0], op=OP.mult)
                        e.tensor_tensor(m2, cim, p_t[1], op=OP.mult)
                        e.tensor_tensor(coil[0], m1, m2, op=OP.subtract)
                        m3 = T(wk, [128, 512], "m3")
                        m4 = T(wk, [128, 512], "m4")
                        e.tensor_tensor(m3, cr, p_t[1], op=OP.mult)
                        e.tensor_tensor(m4, cim, p_t[0], op=OP.mult)
                        e.tensor_tensor(coil[1], m3, m4, op=OP.add)

                        Tt = [T(wk, [128, 512], f"T{i}", FPR) for i in range(2)]
                        ps1 = dft_matmuls(coil, FWD)
                        for pl in range(2):
                            nc.scalar.copy(Tt[pl], ps1[pl])
                        Zt = [T(wk, [128, 512], f"Z{i}", FPR) for i in range(2)]
                        ps2 = dft_matmuls(Tt, FWD)
                        for pl in range(2):
                            nc.vector.tensor_tensor(Zt[pl], ps2[pl], mask[:],
                                                    op=OP.mult)
                        Ut = [T(wk, [128, 512], f"U{i}", FPR) for i in range(2)]
                        ps3 = dft_matmuls(Zt, INV)
                        for pl in range(2):
                            nc.scalar.copy(Ut[pl], ps3[pl])
                        ps4 = dft_matmuls(Ut, INV)
                        sl = ci * 2
                        nc.vector.tensor_tensor(
                            prodA[:, sl * 512:(sl + 1) * 512], cr, ps4[0],
                            op=OP.mult)
                        nc.vector.tensor_tensor(
                            prodA[:, (sl + 1) * 512:(sl + 2) * 512], cim, ps4[1],
                            op=OP.mult)
                        nc.vector.tensor_tensor(
                            prodB[:, sl * 512:(sl + 1) * 512], cr, ps4[1],
                            op=OP.mult)
                        nc.vector.scalar_tensor_tensor(
                            prodB[:, (sl + 1) * 512:(sl + 2) * 512], cim, -1.0,
                            ps4[0], op0=OP.mult, op1=OP.mult)
                    nslot = 2 * ncoil_h
                    for j, prod in enumerate((prodA, prodB)):
                        red = prod.rearrange("p (s w) -> p w s", s=8)[:, :, 0:nslot]
                        nc.vector.tensor_reduce(accs[half][j], red,
                                                axis=AX.X, op=OP.add)
                for i in range(2):
                    nc.vector.scalar_tensor_tensor(
                        ap_t[i], p_t[i], lam128[:], accs[0][i],
                        op0=OP.mult, op1=OP.add)
                    for h2 in range(1, nhalf):
                        nc.vector.tensor_tensor(ap_t[i], ap_t[i], accs[h2][i],
                                                op=OP.add)

                # --- alpha = rTr / <p, Ap> ---
                denom = allreduce_scalar(dot_partial(p_t, ap_t))
                rec = T(sc, [128, 1], "rec")
                nc.vector.reciprocal(rec, denom)
                alpha = T(sc, [128, 1], "alpha")
                nc.vector.tensor_tensor(alpha, rec, rtr128, op=OP.mult)
                for i in range(2):
                    nc.vector.scalar_tensor_tensor(
                        x_t[i], p_t[i], alpha[:], x_t[i], op0=OP.mult, op1=OP.add)
                if not last_iter:
                    nalpha = T(sc, [128, 1], "nalpha")
                    nc.vector.tensor_scalar_mul(nalpha, alpha, -1.0)
                    for i in range(2):
                        nc.vector.scalar_tensor_tensor(
                            r_t[i], ap_t[i], nalpha[:], r_t[i],
                            op0=OP.mult, op1=OP.add)

            if not last_iter:
                rtrnew = allreduce_scalar(dot_partial(r_t, r_t))
                if not first:
                    rec2 = T(sc, [128, 1], "rec2")
                    nc.vector.reciprocal(rec2, rtr128)
                    beta = T(sc, [128, 1], "beta")
                    nc.vector.tensor_tensor(beta, rec2, rtrnew, op=OP.mult)
                    for i in range(2):
                        nc.vector.scalar_tensor_tensor(
                            p_t[i], p_t[i], beta[:], r_t[i],
                            op0=OP.mult, op1=OP.add)
                rtr128 = rtrnew

        # ---- output: x tall-split -> natural [2, 256*256] ----
        for i in range(2):
            nc.sync.dma_start(
                out=dout[i].rearrange("(t p w) -> p t w", t=2, p=128, w=256),
                in_=x_t[i].rearrange("p (t w) -> p t w", t=2))

    if evsem:
        import bass_rust as _bass_rust
        _bass_rust.generate_event_semaphores(nc)
        mybir.codegen_inst_isa_subclasses(nc)
    return nc


# ------------------------------------------------------------------ runner

_CACHE = {}


def _get_nc(key=(N_CG, NCOIL, N_CORES, True)):
    if key not in _CACHE:
        _CACHE[key] = build_nc(*key)
    return _CACHE[key]


def make_in_maps(inputs):
    shared = _prep_shared(
        inputs["w1"], inputs["b1"], inputs["w2"], inputs["b2"], inputs["w3"],
        inputs["b3"], inputs["w4"], inputs["b4"], inputs["w5"], inputs["b5"],
        inputs["lam"])
    in_maps = []
    for b in range(N_CORES):
        m = dict(shared)
        m.update(_prep_core(
            np.asarray(inputs["atb"][b], np.float32),
            np.asarray(inputs["csm_real"][b], np.float32),
            np.asarray(inputs["csm_imag"][b], np.float32),
            np.asarray(inputs["mask"][b], np.float32)))
        in_maps.append(m)
    return in_maps


def run(inputs, trace=False, **kw):
    nc = _get_nc()
    in_maps = make_in_maps(inputs)
    res = run_bass_kernel_spmd(nc, in_maps, core_ids=list(range(N_CORES)),
                               trace=trace, **kw)
    out = np.stack([np.asarray(r["out"]).reshape(2, 256, 256)
                    for r in res.results]).astype(np.float32)
    return out, res


def kernel(**inputs):
    out, _ = run(inputs, trace=False)
    return out

